# revision 16
# baseline (speedup 1.0000x reference)
"""CNF forward (vector field + exact Jacobian trace) on 8 TRN2 cores.

Math: reference computes, per sample x (row of state[:, 1:]):
    f(x)  = W3^T tanh(W2^T tanh(W1^T [x; t] + b1) + b2) + b3      (dx)
    trJ   = trace(df/dx)                                          (aug = -trJ)

Closed form of the trace (instead of D=64 JVPs per sample):
    h1 = tanh([x;t] @ W1 + b1),  h2 = tanh(h1 @ W2 + b2)
    s1 = 1 - h1^2,               s2 = 1 - h2^2
    trJ = s1^T F s2   with  F[h',h] = W2[h',h] * (W3 @ W1[:D])[h, h']

Sharding: data-parallel, 128 samples per core, weights replicated.

All matmul operands in bf16 (fp32 matmuls execute as LOW/HIGH two-pass
on TRN2 — 4x the PE time), fp32 PSUM accumulation. The bias
b1_eff = b1 + t*W1[D] rides as an extra contraction row. Inputs ship
in packed blobs on the SP HWDGE queue, ordered so each consumer's
piece completes just before it is needed. Dummy matmuls on a
random-filled tile bridge the DMA wait gap-free so the PE HAM
clock-gate reaches 2.4 GHz before the real matmuls start (the monitor
tracks real datapath activity: all-zero dummy data does not register,
and any PE-idle hole before the first warm transition resets the
window). trJ and dx leave through separate DMA queues.
"""

import numpy as np
import ml_dtypes

import concourse.bacc as bacc
import concourse.bass as bass
import concourse.tile as tile
from concourse import mybir
from concourse.bass_utils import run_bass_kernel_spmd
from concourse.masks import make_identity
from concourse.tile_rust import add_dep_helper

B, D, H = 1024, 64, 512
NCORES = 8
BC = B // NCORES  # 128 samples per core
KT = H // 128     # 4 feature tiles of 128
DA = D + 1        # augmented contraction (x rows + ones/bias row)
HH = H // 2
F32 = mybir.dt.float32
BF16 = mybir.dt.bfloat16
AF = mybir.ActivationFunctionType
ALU = mybir.AluOpType
ts = bass.ts
BFNP = ml_dtypes.bfloat16

N_WARM = 28  # N=128 dummy matmuls bridging the DMA wait

# blobA column layout: [stT_aug | w1a | w3Ta], padded to 128 partitions
A_ST, A_W1, A_W3T = 0, BC, BC + H
A_COLS = BC + 2 * H
# blobB column layout (two DMA pieces):
#   piece 1: [w2_0 | w2_1 | w3_0..3]   cols [0, 1280)
#   piece 2: [w2_2 | w2_3]             cols [1280, 2304)
B_W2 = [0, 512, 1280, 1792]
B_W3 = 1024
B_SPLIT = 1280
B_COLS = 2304

_NC = {}


def _build(with_bias23: bool):
    """with_bias23: rank-1 bias matmuls for nonzero b2/b3 (setup_inputs()
    has zero biases, so the fast path skips them)."""
    nc = bacc.Bacc()

    blobA = nc.declare_dram_parameter("blobA", [128, A_COLS], BF16,
                                      isOutput=False)
    blobB = nc.declare_dram_parameter("blobB", [128, B_COLS], BF16,
                                      isOutput=False)
    if with_bias23:
        b2r = nc.declare_dram_parameter("b2r", [1, H], BF16, isOutput=False)
        b3r = nc.declare_dram_parameter("b3r", [1, D], BF16, isOutput=False)
    out = nc.declare_dram_parameter("out", [BC, D + 1], F32, isOutput=True)

    with tile.TileContext(nc) as tc:
        with (
            tc.tile_pool(name="const", bufs=1) as cp,
            tc.tile_pool(name="act", bufs=1) as ap,
            tc.tile_pool(name="ps", bufs=1, space="PSUM") as ps,
        ):
            # ---------------- loads (all on the SP HWDGE queue; the Act
            # HWDGE queue starts moving data ~1.7us after issue vs ~0.8us
            # for SP) ----------------
            ba = cp.tile([128, A_COLS], BF16, tag="ba")
            nc.sync.dma_start(out=ba, in_=blobA[:, :])
            bb = cp.tile([128, B_COLS], BF16, tag="bb")
            nc.sync.dma_start(out=bb[:, 0:B_SPLIT], in_=blobB[:, 0:B_SPLIT])
            nc.sync.dma_start(out=bb[:, B_SPLIT:B_COLS],
                              in_=blobB[:, B_SPLIT:B_COLS])
            if with_bias23:
                b2r_sb = cp.tile([1, H], BF16, tag="b2r")
                nc.scalar.dma_start(out=b2r_sb, in_=b2r[:, :])
                b3r_sb = cp.tile([1, D], BF16, tag="b3r")
                nc.scalar.dma_start(out=b3r_sb, in_=b3r[:, :])
                onesr = cp.tile([1, BC], BF16, tag="onesr")
                nc.vector.memset(onesr, 1.0)
            ident = cp.tile([128, 128], BF16, tag="ident")
            make_identity(nc, ident)
            wsrc = ident  # nonzero data: the HAM ignores all-zero streams

            stT = ba[0:DA, A_ST:A_ST + BC]        # [65, BC] (row 64 = ones)
            w1a = ba[0:DA, A_W1:A_W1 + H]         # [65, H]  (row 64 = b1_eff)
            w3Ta = ba[0:DA, A_W3T:A_W3T + H]      # [65, H]  (row 64 = 0)

            # -------- PE warm-up: junk matmuls while DMAs are in flight ----
            # (tag shared with the far-away o_ps bank; results never read)
            warm_ps = ps.tile([128, 128], F32, tag="o", bufs=1, name="warm_ps")
            warm_mm = None
            for i in range(N_WARM):
                warm_mm = nc.tensor.matmul(warm_ps, wsrc, wsrc,
                                           start=True, stop=True)

            # ------------- layer 1 (feature-major): h1T -------------
            # all 4 column-tiles land in ONE psum bank (disjoint column
            # ranges; each matmul is its own start/stop group) so no ACT
            # round-trip gates the z1 stream, then a single tanh.
            h1 = ap.tile([128, H], BF16, tag="h1")   # col-tile j = h1T tile j
            z1_ps = ps.tile([128, KT * BC], F32, tag="z1", bufs=1)
            z1_mm = []
            for j in range(KT):
                z1_mm.append(
                    nc.tensor.matmul(z1_ps[:, ts(j, BC)], w1a[:, ts(j, 128)],
                                     stT, start=True, stop=True))
                if j == 0:
                    add_dep_helper(z1_mm[0].ins, warm_mm.ins, sync=False,
                                   reason="pe-order z1 after warmup")
            nc.scalar.activation(h1, z1_ps, AF.Tanh)
            s1 = ap.tile([128, H], BF16, tag="s1")
            nc.gpsimd.tensor_mul(s1, h1, h1)
            nc.gpsimd.tensor_scalar(s1, s1, -1.0, 1.0, ALU.mult, ALU.add)

            # ------------- trace weight matrix F (weights only) -------------
            # E2T tile m = (W1[:D] tile m)^T @ W3^T; bias row x zero row = 0.
            # four separate psum banks so the e2t stream never waits on the
            # DVE F-multiplies
            f_sb = []
            for m in range(KT):
                e2t_ps = ps.tile([128, H], F32, tag=f"e2t_{m}", bufs=1,
                                 name=f"e2t_ps_{m}")
                e2t_mm = nc.tensor.matmul(e2t_ps, w1a[:, ts(m, 128)], w3Ta,
                                          start=True, stop=True)
                if m == 0:
                    add_dep_helper(e2t_mm.ins, z1_mm[KT - 1].ins, sync=False,
                                   reason="pe-order e2t after z1")
                fm = ap.tile([128, H], BF16, tag=f"f_{m}")
                nc.vector.tensor_mul(fm, bb[:, B_W2[m]:B_W2[m] + H], e2t_ps)
                f_sb.append(fm)

            # ------------- layer 2 (batch-major): h2, s2 -------------
            # z2 reuses the z1 bank (WAR: waits for the h1 tanh read)
            z2_ps = ps.tile([BC, H], F32, tag="z1", bufs=1, name="z2_ps")
            for k in range(KT):
                mm = nc.tensor.matmul(z2_ps, h1[:, ts(k, 128)],
                                      bb[:, B_W2[k]:B_W2[k] + H],
                                      start=(k == 0),
                                      stop=(k == KT - 1 and not with_bias23))
                if k == 0:
                    add_dep_helper(mm.ins, z1_mm[KT - 1].ins, sync=False,
                                   reason="pe-order z2 after z1")
            if with_bias23:
                nc.tensor.matmul(z2_ps, onesr, b2r_sb, start=False, stop=True)
            h2 = ap.tile([BC, H], BF16, tag="h2")
            for j in range(2):
                nc.scalar.activation(h2[:, ts(j, HH)], z2_ps[:, ts(j, HH)],
                                     AF.Tanh)
            # s2 halves: low half on DVE (free once the F multiplies drain),
            # high half on gpsimd (free once s1 is done)
            s2 = ap.tile([BC, H], BF16, tag="s2")
            nc.vector.tensor_mul(s2[:, 0:HH], h2[:, 0:HH], h2[:, 0:HH])
            nc.vector.tensor_scalar(s2[:, 0:HH], s2[:, 0:HH], -1.0, 1.0,
                                    ALU.mult, ALU.add)
            nc.gpsimd.tensor_mul(s2[:, HH:H], h2[:, HH:H], h2[:, HH:H])
            nc.gpsimd.tensor_scalar(s2[:, HH:H], s2[:, HH:H], -1.0, 1.0,
                                    ALU.mult, ALU.add)

            # ------------- trJ = s1^T F s2 (batch-major) -------------
            t2_ps = ps.tile([BC, H], F32, tag="t2", bufs=1)
            for k in range(KT):
                nc.tensor.matmul(t2_ps, s1[:, ts(k, 128)], f_sb[k],
                                 start=(k == 0), stop=(k == KT - 1))
            final_sb = ap.tile([BC, D + 1], F32, tag="final")
            ttr_scr = ap.tile([BC, H], F32, tag="ttr_scr")
            tr_par = ap.tile([BC, 2], F32, tag="tr_par")
            for hf in range(2):
                sl = slice(hf * HH, (hf + 1) * HH)
                # fused: scratch = (t2 * -1) * s2, partial = sum(scratch)
                nc.vector.scalar_tensor_tensor(
                    out=ttr_scr[:, sl], in0=t2_ps[:, sl], scalar=-1.0,
                    in1=s2[:, sl], op0=ALU.mult, op1=ALU.mult,
                    accum_out=tr_par[:, hf:hf + 1])
            nc.vector.tensor_reduce(out=final_sb[:, 0:1], in_=tr_par,
                                    op=ALU.add, axis=mybir.AxisListType.X)
            # trJ leaves via the Act queue, in parallel with dx on SP below
            nc.scalar.dma_start(out=out[:, 0:1], in_=final_sb[:, 0:1])

            # ------------- layer 3 (batch-major): dx -------------
            h2T_sb = []
            for j in range(KT):
                hT_ps = ps.tile([128, BC], BF16, tag="hT", bufs=1,
                                name=f"hT_ps_{j}")
                nc.tensor.transpose(hT_ps, h2[:, ts(j, 128)], ident)
                hT = ap.tile([128, BC], BF16, tag=f"h2T_{j}", name=f"hT_{j}")
                nc.scalar.copy(hT, hT_ps)
                h2T_sb.append(hT)
            o_ps = ps.tile([BC, D], F32, tag="o", bufs=1)
            for k in range(KT):
                nc.tensor.matmul(o_ps, h2T_sb[k],
                                 bb[:, B_W3 + k * D:B_W3 + (k + 1) * D],
                                 start=(k == 0),
                                 stop=(k == KT - 1 and not with_bias23))
            if with_bias23:
                nc.tensor.matmul(o_ps, onesr, b3r_sb, start=False, stop=True)
            nc.scalar.copy(final_sb[:, 1:D + 1], o_ps)
            nc.sync.dma_start(out=out[:, 1:D + 1], in_=final_sb[:, 1:D + 1])

    nc.finalize()
    return nc


def _get_nc(with_bias23: bool):
    key = bool(with_bias23)
    if key not in _NC:
        _NC[key] = _build(key)
    return _NC[key]


def make_in_maps(inputs):
    f32 = lambda a: np.ascontiguousarray(np.asarray(a), dtype=np.float32)
    bf = lambda a: np.ascontiguousarray(np.asarray(a, dtype=np.float32)
                                        ).astype(BFNP)
    state = f32(inputs["state"])
    t = float(np.asarray(inputs["t"]).reshape(-1)[0])
    W1 = f32(inputs["W1"])
    b1 = f32(inputs["b1"]).reshape(H)
    W2 = f32(inputs["W2"])
    b2 = f32(inputs["b2"]).reshape(H)
    W3 = f32(inputs["W3"])
    b3 = f32(inputs["b3"]).reshape(D)

    with_bias23 = bool(np.any(b2) or np.any(b3))

    b1_eff = b1 + t * W1[D]
    w1a = np.concatenate([W1[:D], b1_eff[None, :]], axis=0)       # [65, H]
    w3Ta = np.concatenate([W3.T, np.zeros((1, H), np.float32)], axis=0)
    aw = bf(np.concatenate([w1a, w3Ta], axis=1))                  # [65, 2H]

    w2t = [bf(W2[k * 128:(k + 1) * 128]) for k in range(KT)]
    w3t = bf(np.concatenate(
        [W3[k * 128:(k + 1) * 128] for k in range(KT)], axis=1))  # [128, 256]
    blobB = np.ascontiguousarray(np.concatenate(
        [w2t[0], w2t[1], w3t, w2t[2], w2t[3]], axis=1))

    base = {"blobB": blobB}
    if with_bias23:
        base["b2r"] = bf(b2.reshape(1, H))
        base["b3r"] = bf(b3.reshape(1, D))
    in_maps = []
    ones_row = np.ones((1, BC), np.float32)
    pad = np.zeros((128 - DA, A_COLS), BFNP)
    for c in range(NCORES):
        m = dict(base)
        stT_aug = np.concatenate(
            [state[c * BC:(c + 1) * BC, 1:].T, ones_row], axis=0)  # [65, BC]
        top = np.concatenate([bf(stT_aug), aw], axis=1)            # [65, 1152]
        m["blobA"] = np.ascontiguousarray(np.concatenate([top, pad], axis=0))
        in_maps.append(m)
    return with_bias23, in_maps


def kernel(**inputs) -> np.ndarray:
    with_bias23, in_maps = make_in_maps(inputs)
    res = run_bass_kernel_spmd(_get_nc(with_bias23), in_maps,
                               list(range(NCORES))).results
    return np.concatenate([res[c]["out"] for c in range(NCORES)], axis=0)
